# revision 1
# baseline (speedup 1.0000x reference)
"""Causal self-attention with RoPE on 8 Trainium2 NeuronCores.

Sharding: batch x head-group. Core c handles batch b = c//2 and head group
g = c%2 (8 of 16 heads). Each core runs the full per-(batch, head-group)
pipeline on device:

  QKV^T projection -> RoPE -> causal flash-style attention -> partial
  output projection (its heads' slice of W_out rows).

The host sums the two partial projections per batch and adds b_out.

Device layout choices (all matmuls contract over the partition dim):
  - x is fed pre-transposed (xT: [D, L]) so Q^T/K^T = W^T x^T come out with
    head dims on partitions, which is exactly the lhsT/rhs layout the
    score matmul S^T = K Q^T wants.  V is computed in natural [L, dv]
    layout (lhsT = xT tile), which is the lhsT layout the PV matmul wants.
  - S^T = matmul(lhsT=K^T tile, rhs=Q^T tile) comes out [lk, lq]; exp(S^T)
    is then directly the lhsT-side...  actually rhs of the PV matmul:
    Y^T = matmul(lhsT=V_aug, rhs=expS^T).  A ones column appended to V
    yields the softmax denominator for free in row 64 of the PV psum.
  - Softmax uses no max subtraction: scores are O(1) here (|s|/sqrt(dh)
    stays far below fp32/bf16 exp range), so exp/sum/divide is exact.
  - Causal masking is multiplicative on exp(S^T) (0/1 mask slices), only
    needed on the 4 diagonal 128-tiles of each 512-wide query chunk.
"""

import os
import sys

if "/opt/trn_rl_repo" not in sys.path:
    sys.path.insert(0, "/opt/trn_rl_repo")

import numpy as np
import ml_dtypes

import concourse.bass as bass
import concourse.mybir as mybir
import concourse.tile as tile

F32 = mybir.dt.float32
F32R = mybir.dt.float32r
BF16 = mybir.dt.bfloat16

B, L, D = 4, 2048, 1024
H, DH = 16, 64
NCORES = 8
G = 2                 # head groups (cores per batch)
HPC = H // G          # heads per core = 8
DQ = HPC * DH         # per-core q/k/v width = 512
PAIRS = HPC // 2      # 128-partition head pairs = 4
CHUNK = 512           # query-chunk (matmul free dim)
NCH = L // CHUNK      # 4
KT = D // 128         # 8 k-tiles over d_model
LT = L // 128         # 16 l-tiles
VW = DH + 1           # V columns per head incl. ones column = 65

LAST_RESULTS = None   # test harness reads perf fields from here


def legalize_bir_waits(bir_json: bytes) -> bytes:
    """Split multi-wait sync_infos into standalone EventSemaphore instrs.

    This container's walrus codegen accepts at most ONE sync wait per
    instruction (two for EventSemaphore), but Tile's sem assigner happily
    attaches several.  For every instruction carrying N>1 waits, keep one
    and hoist the rest onto EventSemaphore instructions inserted directly
    before it on the same engine (same block), which preserves each
    engine's program order and therefore the sync semantics.
    """
    import json as _json

    j = _json.loads(bir_json)
    uid = [0]
    for fn in j["functions"]:
        for blk in fn["blocks"]:
            out_insts = []
            for inst in blk["instructions"]:
                si = inst.get("sync_info")
                waits = (si or {}).get("on_wait") or []
                cap = 2 if inst.get("opcode") == "EventSemaphore" else 1
                if len(waits) > cap:
                    extra, keep = waits[:-cap], waits[-cap:]
                    for i in range(0, len(extra), 2):
                        uid[0] += 1
                        out_insts.append(
                            {
                                "name": f"antwaitfix-{uid[0]}",
                                "opcode": "EventSemaphore",
                                "engine": inst["engine"],
                                "ins": [],
                                "outs": [],
                                "debug": inst.get("debug", 0),
                                "sync_info": {
                                    "on_wait": extra[i : i + 2],
                                    "on_update": [],
                                },
                            }
                        )
                    si["on_wait"] = keep
                out_insts.append(inst)
            blk["instructions"] = out_insts
    return _json.dumps(j).encode()


def build_module():
    nc = bass.Bass(use_seq_codegen=True)

    xT = nc.declare_dram_parameter("xT", [D, L], BF16, isOutput=False)
    wq = nc.declare_dram_parameter("wq", [D, DQ], BF16, isOutput=False)
    wk = nc.declare_dram_parameter("wk", [D, DQ], BF16, isOutput=False)
    wv = nc.declare_dram_parameter("wv", [D, DQ], BF16, isOutput=False)
    wo = nc.declare_dram_parameter("wo", [DQ, D], BF16, isOutput=False)
    bq = nc.declare_dram_parameter("bq", [128, PAIRS], F32, isOutput=False)
    bk = nc.declare_dram_parameter("bk", [128, PAIRS], F32, isOutput=False)
    bv = nc.declare_dram_parameter("bv", [128, DQ], F32, isOutput=False)
    cosT = nc.declare_dram_parameter("cosT", [128, L], BF16, isOutput=False)
    sinT = nc.declare_dram_parameter("sinT", [128, L], BF16, isOutput=False)
    maskb = nc.declare_dram_parameter("maskb", [128, 896], BF16, isOutput=False)
    out = nc.declare_dram_parameter("out", [L, D], F32, isOutput=True)

    with tile.TileContext(nc) as tc:
        with (
            tc.tile_pool(name="const", bufs=1) as cp,
            tc.tile_pool(name="acts", bufs=1) as ap,
            tc.tile_pool(name="work", bufs=5) as wp,
            tc.tile_pool(name="pss", bufs=4, space="PSUM") as pss,
            tc.tile_pool(name="psy", bufs=4, space="PSUM") as psy,
        ):
            # ---- constant / activation loads (split for DMA-queue spread)
            xT_sb = ap.tile([128, KT, L], BF16)
            for kt in range(KT):
                nc.sync.dma_start(
                    xT_sb[:, kt, :],
                    xT.rearrange("(kt p) l -> p kt l", p=128)[:, kt, :],
                )
            wq_sb = cp.tile([128, KT, DQ], BF16)
            wk_sb = cp.tile([128, KT, DQ], BF16)
            wv_sb = cp.tile([128, KT, DQ], BF16)
            for kt in range(KT):
                nc.sync.dma_start(
                    wq_sb[:, kt, :], wq.rearrange("(kt p) m -> p kt m", p=128)[:, kt, :]
                )
                nc.sync.dma_start(
                    wk_sb[:, kt, :], wk.rearrange("(kt p) m -> p kt m", p=128)[:, kt, :]
                )
                nc.sync.dma_start(
                    wv_sb[:, kt, :], wv.rearrange("(kt p) m -> p kt m", p=128)[:, kt, :]
                )
            wo_sb = cp.tile([128, PAIRS, D], BF16)
            for pr in range(PAIRS):
                nc.sync.dma_start(
                    wo_sb[:, pr, :], wo.rearrange("(pr p) c -> p pr c", p=128)[:, pr, :]
                )
            bq_sb = cp.tile([128, PAIRS], F32)
            bk_sb = cp.tile([128, PAIRS], F32)
            bv_sb = cp.tile([128, DQ], F32)
            cos_sb = cp.tile([128, L], BF16)
            sin_sb = cp.tile([128, L], BF16)
            mask_sb = cp.tile([128, 896], BF16)
            nc.sync.dma_start(bq_sb[:], bq[:])
            nc.sync.dma_start(bk_sb[:], bk[:])
            nc.sync.dma_start(bv_sb[:], bv[:])
            nc.sync.dma_start(cos_sb[:], cosT[:])
            nc.sync.dma_start(sin_sb[:], sinT[:])
            nc.sync.dma_start(mask_sb[:], maskb[:])
            # memset can't encode a float32r immediate; memset f32 then
            # copy-convert (bitwise identical) into the f32r tile.
            ones_f32 = cp.tile([128, 64], F32)
            nc.vector.memset(ones_f32[:], 1.0)
            ones_sb = cp.tile([128, 64], F32R)
            with nc.allow_low_precision(reason="f32r ones for bcast mm"):
                nc.vector.tensor_copy(ones_sb[:], ones_f32[:])

            qT_sb = ap.tile([128, PAIRS, L], BF16)
            kT_sb = ap.tile([128, PAIRS, L], BF16)
            v_sb = ap.tile([128, LT, HPC * VW], BF16)
            yT_sb = ap.tile([128, PAIRS, L], BF16)

            # ---- phase 1: QKV projection
            for mt in range(PAIRS):
                for c in range(NCH):
                    for dst, w_sb, b_sb in ((qT_sb, wq_sb, bq_sb), (kT_sb, wk_sb, bk_sb)):
                        ps = pss.tile([128, CHUNK], F32, tag="ps")
                        for kt in range(KT):
                            nc.tensor.matmul(
                                ps[:],
                                w_sb[:, kt, mt * 128 : (mt + 1) * 128],
                                xT_sb[:, kt, c * CHUNK : (c + 1) * CHUNK],
                                start=(kt == 0),
                                stop=(kt == KT - 1),
                            )
                        nc.scalar.activation(
                            dst[:, mt, c * CHUNK : (c + 1) * CHUNK],
                            ps[:],
                            mybir.ActivationFunctionType.Identity,
                            bias=b_sb[:, mt : mt + 1],
                        )
            for lt in range(LT):
                ps = pss.tile([128, CHUNK], F32, tag="ps")
                for kt in range(KT):
                    nc.tensor.matmul(
                        ps[:],
                        xT_sb[:, kt, lt * 128 : (lt + 1) * 128],
                        wv_sb[:, kt, :],
                        start=(kt == 0),
                        stop=(kt == KT - 1),
                    )
                vdst = v_sb[:, lt, :].rearrange("p (h c) -> p h c", c=VW)
                nc.vector.tensor_add(vdst[:, :, 0:DH], ps[:], bv_sb[:])
                nc.vector.memset(vdst[:, :, DH:VW], 1.0)

            # ---- phase 1.5: RoPE on Q^T and K^T (in place)
            for dst in (qT_sb, kT_sb):
                for mt in range(PAIRS):
                    t = dst[:, mt, :]
                    swp = wp.tile([128, L], BF16, tag="swp")
                    for i in range(4):
                        j = i ^ 1
                        nc.sync.dma_start(
                            swp[i * 32 : (i + 1) * 32, :], t[j * 32 : (j + 1) * 32, :]
                        )
                    nc.vector.tensor_mul(swp[:], swp[:], sin_sb[:])
                    nc.vector.tensor_mul(t, t, cos_sb[:])
                    nc.vector.tensor_add(t, t, swp[:])

            # ---- phase 2: attention (chunk-outer; the two pairs of each
            # half interleave at the kt level so PE always has independent
            # score/PV work while ACT runs the other pair's exp)
            for c in range(NCH):
                q0 = c * CHUNK
                n_lk = (q0 + CHUNK) // 128
                for half in range(2):
                    prs = (2 * half, 2 * half + 1)
                    ys = {
                        pr: [
                            psy.tile(
                                [128, CHUNK], F32, tag="psy",
                                name=f"psy_{pr}_{c}_{i}",
                            )
                            for i in range(2)
                        ]
                        for pr in prs
                    }
                    for kt in range(n_lk):
                        k0 = kt * 128
                        for pr in prs:
                            kT_p = kT_sb[:, pr, :]
                            qT_p = qT_sb[:, pr, :]
                            exps = []
                            for hh in range(2):
                                ps = pss.tile([128, CHUNK], F32, tag="ps")
                                nc.tensor.matmul(
                                    ps[:],
                                    kT_p[hh * 64 : (hh + 1) * 64, k0 : k0 + 128],
                                    qT_p[hh * 64 : (hh + 1) * 64, q0 : q0 + CHUNK],
                                    start=True,
                                    stop=True,
                                )
                                ex = wp.tile([128, CHUNK], BF16, tag="exp")
                                nc.scalar.activation(
                                    ex[:], ps[:], mybir.ActivationFunctionType.Exp,
                                    scale=float(1.0 / np.sqrt(DH)),
                                )
                                if k0 >= q0:
                                    s = 384 - (k0 - q0)
                                    nc.vector.tensor_mul(
                                        ex[:], ex[:], mask_sb[:, s : s + CHUNK]
                                    )
                                exps.append(ex)
                            for hh in range(2):
                                h = 2 * pr + hh
                                nc.tensor.matmul(
                                    ys[pr][hh][0:VW, :],
                                    v_sb[:, kt, h * VW : (h + 1) * VW],
                                    exps[hh][:],
                                    start=(kt == 0),
                                    stop=(kt == n_lk - 1),
                                )
                    for pr in prs:
                        for hh in range(2):
                            den = wp.tile([128, CHUNK], F32R, tag="den")
                            with nc.allow_low_precision(reason="f32r recip"):
                                nc.vector.reciprocal(
                                    den[64:65, :], ys[pr][hh][64:65, :]
                                )
                            bc = pss.tile([128, CHUNK], F32, tag="ps")
                            nc.tensor.matmul(
                                bc[0:64, :],
                                ones_sb[64:65, :],
                                den[64:65, :],
                                start=True,
                                stop=True,
                            )
                            # DVE has a single PSUM port: stage the broadcast
                            # through SBUF (ScalarE copy) so the multiply
                            # reads only one PSUM operand.
                            bcs = wp.tile([64, CHUNK], F32, tag="bcs")
                            nc.scalar.copy(bcs[:], bc[0:64, :])
                            if hh == 0:
                                nc.vector.tensor_mul(
                                    yT_sb[0:64, pr, q0 : q0 + CHUNK],
                                    ys[pr][hh][0:64, :],
                                    bcs[:],
                                )
                            else:
                                # walrus rejects elementwise ops whose out/in
                                # partition bases differ; base-0 temp + DMA
                                # does the partition move.
                                yt = wp.tile([64, CHUNK], BF16, tag="ytmp")
                                nc.vector.tensor_mul(
                                    yt[:], ys[pr][hh][0:64, :], bcs[:]
                                )
                                nc.sync.dma_start(
                                    yT_sb[64:128, pr, q0 : q0 + CHUNK], yt[:]
                                )

                # ---- phase 3 (interleaved): output projection for this
                # chunk's l-tiles, partial over this core's W_out rows
                for lt in range(4 * c, 4 * c + 4):
                    for cc in range(2):
                        ps = pss.tile([128, CHUNK], F32, tag="ps")
                        for pr in range(PAIRS):
                            nc.tensor.matmul(
                                ps[:],
                                yT_sb[:, pr, lt * 128 : (lt + 1) * 128],
                                wo_sb[:, pr, cc * CHUNK : (cc + 1) * CHUNK],
                                start=(pr == 0),
                                stop=(pr == PAIRS - 1),
                            )
                        ob = wp.tile([128, CHUNK], F32, tag="ob")
                        nc.vector.tensor_copy(ob[:], ps[:])
                        nc.sync.dma_start(
                            out[
                                lt * 128 : (lt + 1) * 128,
                                cc * CHUNK : (cc + 1) * CHUNK,
                            ],
                            ob[:],
                        )
    return nc


def _rope_tables():
    inv_freq = (1.0 / (10000.0 ** (np.arange(0, DH, 2, dtype=np.float32) / DH))).astype(
        np.float32
    )
    t = np.arange(L, dtype=np.float32)
    freqs = np.einsum("l,d->ld", t, inv_freq).astype(np.float32)  # (L, 32)
    emb = np.concatenate([freqs, freqs], axis=-1)                 # (L, 64)
    cos = np.cos(emb).astype(np.float32)
    sin = np.sin(emb).astype(np.float32)
    cosT = cos.T                                   # (64, L)
    sinT = sin.T.copy()
    sinT[0:32] = -sinT[0:32]                       # fold rotate_half sign
    cos128 = np.tile(cosT, (2, 1))                 # (128, L)
    sin128 = np.tile(sinT, (2, 1))
    return cos128, sin128


def _mask_big():
    # maskb[p, j] = 1.0 iff p <= j - 384 (slice at s = 384-delta gives the
    # diagonal-tile mask "p <= f - delta")
    p = np.arange(128)[:, None]
    j = np.arange(896)[None, :]
    return (p <= j - 384).astype(np.float32)


def _bf16(a):
    return np.asarray(a, dtype=np.float32).astype(ml_dtypes.bfloat16)


_COMPILED = None


def kernel(x, pad_mask, W_qkv, b_qkv, W_out, b_out):
    global LAST_RESULTS, _COMPILED
    from concourse.bass_utils import run_bass_kernel_spmd

    x = np.asarray(x, dtype=np.float32)
    W_qkv = np.asarray(W_qkv, dtype=np.float32)
    b_qkv = np.asarray(b_qkv, dtype=np.float32)
    W_out = np.asarray(W_out, dtype=np.float32)
    b_out = np.asarray(b_out, dtype=np.float32)

    cos128, sin128 = _rope_tables()
    maskb = _mask_big()

    in_maps = []
    for core in range(NCORES):
        b, g = core // G, core % G
        sl = slice(g * DQ, (g + 1) * DQ)
        wq = W_qkv[:, 0 * D : 1 * D][:, sl]
        wk = W_qkv[:, 1 * D : 2 * D][:, sl]
        wv = W_qkv[:, 2 * D : 3 * D][:, sl]
        bqv = b_qkv[0 * D : 1 * D][sl]
        bkv = b_qkv[1 * D : 2 * D][sl]
        bvv = b_qkv[2 * D : 3 * D][sl]
        in_maps.append(
            {
                "xT": _bf16(x[b].T),
                "wq": _bf16(wq),
                "wk": _bf16(wk),
                "wv": _bf16(wv),
                "wo": _bf16(W_out[sl, :]),
                "bq": np.ascontiguousarray(bqv.reshape(PAIRS, 128).T),
                "bk": np.ascontiguousarray(bkv.reshape(PAIRS, 128).T),
                "bv": np.tile(bvv[None, :], (128, 1)).astype(np.float32),
                "cosT": _bf16(cos128),
                "sinT": _bf16(sin128),
                "maskb": _bf16(maskb),
            }
        )

    if _COMPILED is None:
        nc = build_module()
        fixed = legalize_bir_waits(nc.to_json_bytes())
        nc.to_json_bytes = lambda: fixed  # bass2jax ships this BIR to walrus
        _COMPILED = nc
    nc = _COMPILED

    res = run_bass_kernel_spmd(
        nc,
        in_maps,
        core_ids=list(range(NCORES)),
        trace=bool(os.environ.get("BASS_TRACE")),
    )
    LAST_RESULTS = res

    out = np.zeros((B, L, D), dtype=np.float32)
    for core in range(NCORES):
        out[core // G] += np.asarray(res.results[core]["out"], dtype=np.float32)
    out += b_out[None, None, :]
    return out



# revision 4
# speedup vs baseline: 1.6108x; 1.6108x over previous
"""Causal self-attention with RoPE on 8 Trainium2 NeuronCores.

Sharding: batch x head-group. Core c handles batch b = c//2 and head group
g = c%2 (8 of 16 heads). Each core runs the full per-(batch, head-group)
pipeline on device:

  QKV^T projection -> RoPE -> causal flash-style attention -> partial
  output projection (its heads' slice of W_out rows).

The host sums the two partial projections per batch and adds b_out.

v2 changes vs v1 (which measured 572-666us, PE cold-throttled 58% of the
time due to 3-4us PE-idle gaps from serialized 4us reciprocals blocking
PSUM-bank reuse):
  - Causal mask is applied on the TensorEngine: a bf16 [-1e30] triangle is
    accumulated into the diagonal score tiles via an identity-lhsT matmul,
    so exp() emits exact zeros and the DVE mask multiplies disappear.
  - The two heads of a pair write one [128,1024] score PSUM tile (2 banks)
    and share ONE exp ACTIVATE, amortizing ACT's 352-cycle fixed cost.
    The K=64 score matmuls land on disjoint row groups (tile_position
    (0,0)/(64,0)) so they can run concurrently in the PE array.
  - PV matmuls are narrowed past the diagonal (columns left of the
    staircase are fully masked and contribute nothing).
  - Softmax division: PV PSUM is drained to SBUF immediately (frees the
    bank in <1us so the PE never stalls), the 4 denominator rows of a
    pair-pair are DMA-staged to partitions {0,32,64,96} of one tile and
    reciprocal'd in ONE DVE op (lanes are per-partition parallel), then a
    K=1 ones-matmul broadcasts each row and a DVE multiply normalizes.
"""

import os
import sys

if "/opt/trn_rl_repo" not in sys.path:
    sys.path.insert(0, "/opt/trn_rl_repo")

import numpy as np
import ml_dtypes

import concourse.bass as bass
import concourse.mybir as mybir
import concourse.tile as tile

F32 = mybir.dt.float32
F32R = mybir.dt.float32r
BF16 = mybir.dt.bfloat16

B, L, D = 4, 2048, 1024
H, DH = 16, 64
NCORES = 8
G = 2                 # head groups (cores per batch)
HPC = H // G          # heads per core = 8
DQ = HPC * DH         # per-core q/k/v width = 512
PAIRS = HPC // 2      # 128-partition head pairs = 4
CHUNK = 512           # query-chunk (matmul free dim)
NCH = L // CHUNK      # 4
KT = D // 128         # 8 k-tiles over d_model
LT = L // 128         # 16 l-tiles
VW = DH + 1           # V columns per head incl. ones column = 65
NEG = -1.0e30         # causal-mask additive constant (exp -> exact 0)

LAST_RESULTS = None   # test harness reads perf fields from here


def legalize_bir_waits(bir_json: bytes) -> bytes:
    """Split multi-wait sync_infos into standalone EventSemaphore instrs.

    This container's walrus codegen accepts at most ONE sync wait per
    instruction (two for EventSemaphore), but Tile's sem assigner happily
    attaches several.  For every instruction carrying N>1 waits, keep one
    and hoist the rest onto EventSemaphore instructions inserted directly
    before it on the same engine (same block), which preserves each
    engine's program order and therefore the sync semantics.
    """
    import json as _json

    j = _json.loads(bir_json)
    uid = [0]
    for fn in j["functions"]:
        for blk in fn["blocks"]:
            out_insts = []
            for inst in blk["instructions"]:
                si = inst.get("sync_info")
                waits = (si or {}).get("on_wait") or []
                cap = 2 if inst.get("opcode") == "EventSemaphore" else 1
                if len(waits) > cap:
                    extra, keep = waits[:-cap], waits[-cap:]
                    for i in range(0, len(extra), 2):
                        uid[0] += 1
                        out_insts.append(
                            {
                                "name": f"antwaitfix-{uid[0]}",
                                "opcode": "EventSemaphore",
                                "engine": inst["engine"],
                                "ins": [],
                                "outs": [],
                                "debug": inst.get("debug", 0),
                                "sync_info": {
                                    "on_wait": extra[i : i + 2],
                                    "on_update": [],
                                },
                            }
                        )
                    si["on_wait"] = keep
                out_insts.append(inst)
            blk["instructions"] = out_insts
    return _json.dumps(j).encode()


def build_module():
    nc = bass.Bass(use_seq_codegen=True)

    xT = nc.declare_dram_parameter("xT", [D, L], BF16, isOutput=False)
    wq = nc.declare_dram_parameter("wq", [D, DQ], BF16, isOutput=False)
    wk = nc.declare_dram_parameter("wk", [D, DQ], BF16, isOutput=False)
    wv = nc.declare_dram_parameter("wv", [D, DQ], BF16, isOutput=False)
    wo = nc.declare_dram_parameter("wo", [DQ, D], BF16, isOutput=False)
    bq = nc.declare_dram_parameter("bq", [128, PAIRS], F32, isOutput=False)
    bk = nc.declare_dram_parameter("bk", [128, PAIRS], F32, isOutput=False)
    bv = nc.declare_dram_parameter("bv", [128, DQ], F32, isOutput=False)
    cosT = nc.declare_dram_parameter("cosT", [128, L], BF16, isOutput=False)
    sinT = nc.declare_dram_parameter("sinT", [128, L], BF16, isOutput=False)
    trin = nc.declare_dram_parameter("trin", [128, 128], BF16, isOutput=False)
    iden = nc.declare_dram_parameter("iden", [128, 128], BF16, isOutput=False)
    out = nc.declare_dram_parameter("out", [L, D], F32, isOutput=True)

    with tile.TileContext(nc) as tc:
        with (
            tc.tile_pool(name="const", bufs=1) as cp,
            tc.tile_pool(name="acts", bufs=1) as ap,
            tc.tile_pool(name="work", bufs=2) as wp,
            tc.tile_pool(name="psum", bufs=2, space="PSUM") as ps,
        ):
            # ---- constant / activation loads (split for DMA-queue spread)
            xT_sb = ap.tile([128, KT, L], BF16)
            for kt in range(KT):
                nc.sync.dma_start(
                    xT_sb[:, kt, :],
                    xT.rearrange("(kt p) l -> p kt l", p=128)[:, kt, :],
                )
            wq_sb = cp.tile([128, KT, DQ], BF16)
            wk_sb = cp.tile([128, KT, DQ], BF16)
            wv_sb = cp.tile([128, KT, DQ], BF16)
            for kt in range(KT):
                nc.sync.dma_start(
                    wq_sb[:, kt, :], wq.rearrange("(kt p) m -> p kt m", p=128)[:, kt, :]
                )
                nc.sync.dma_start(
                    wk_sb[:, kt, :], wk.rearrange("(kt p) m -> p kt m", p=128)[:, kt, :]
                )
                nc.sync.dma_start(
                    wv_sb[:, kt, :], wv.rearrange("(kt p) m -> p kt m", p=128)[:, kt, :]
                )
            wo_sb = cp.tile([128, PAIRS, D], BF16)
            for pr in range(PAIRS):
                nc.sync.dma_start(
                    wo_sb[:, pr, :], wo.rearrange("(pr p) c -> p pr c", p=128)[:, pr, :]
                )
            bq_sb = cp.tile([128, PAIRS], F32)
            bk_sb = cp.tile([128, PAIRS], F32)
            bv_sb = cp.tile([128, DQ], F32)
            cos_sb = cp.tile([128, L], BF16)
            sin_sb = cp.tile([128, L], BF16)
            tri_sb = cp.tile([128, 128], BF16)
            id_sb = cp.tile([128, 128], BF16)
            nc.sync.dma_start(bq_sb[:], bq[:])
            nc.sync.dma_start(bk_sb[:], bk[:])
            nc.sync.dma_start(bv_sb[:], bv[:])
            nc.sync.dma_start(cos_sb[:], cosT[:])
            nc.sync.dma_start(sin_sb[:], sinT[:])
            nc.sync.dma_start(tri_sb[:], trin[:])
            nc.sync.dma_start(id_sb[:], iden[:])
            # memset can't encode a float32r immediate; memset f32 then
            # copy-convert (bitwise identical) into the f32r tile.
            ones_f32 = cp.tile([128, 64], F32)
            nc.vector.memset(ones_f32[:], 1.0)
            ones_sb = cp.tile([128, 64], F32R)
            with nc.allow_low_precision(reason="f32r ones for bcast mm"):
                nc.vector.tensor_copy(ones_sb[:], ones_f32[:])

            qT_sb = ap.tile([128, PAIRS, L], BF16)
            kT_sb = ap.tile([128, PAIRS, L], BF16)
            v_sb = ap.tile([128, LT, HPC * VW], BF16)
            yT_sb = ap.tile([128, PAIRS, L], BF16)

            # ---- phase 1: QKV projection.  q and k of one (pair, chunk)
            # share a [128,1024] psum tile (2 banks).
            for mt in range(PAIRS):
                for c in range(NCH):
                    qk = ps.tile([128, 1024], F32, tag="sc")
                    for half, w_sb in ((0, wq_sb), (1, wk_sb)):
                        for kt in range(KT):
                            nc.tensor.matmul(
                                qk[:, half * 512 : half * 512 + 512],
                                w_sb[:, kt, mt * 128 : (mt + 1) * 128],
                                xT_sb[:, kt, c * CHUNK : (c + 1) * CHUNK],
                                start=(kt == 0),
                                stop=(kt == KT - 1),
                            )
                    nc.scalar.activation(
                        qT_sb[:, mt, c * CHUNK : (c + 1) * CHUNK],
                        qk[:, 0:512],
                        mybir.ActivationFunctionType.Identity,
                        bias=bq_sb[:, mt : mt + 1],
                    )
                    nc.scalar.activation(
                        kT_sb[:, mt, c * CHUNK : (c + 1) * CHUNK],
                        qk[:, 512:1024],
                        mybir.ActivationFunctionType.Identity,
                        bias=bk_sb[:, mt : mt + 1],
                    )
            for lt in range(LT):
                vps = ps.tile([128, CHUNK], F32, tag="qp")
                for kt in range(KT):
                    nc.tensor.matmul(
                        vps[:],
                        xT_sb[:, kt, lt * 128 : (lt + 1) * 128],
                        wv_sb[:, kt, :],
                        start=(kt == 0),
                        stop=(kt == KT - 1),
                    )
                vdst = v_sb[:, lt, :].rearrange("p (h c) -> p h c", c=VW)
                nc.vector.tensor_add(vdst[:, :, 0:DH], vps[:], bv_sb[:])
                nc.vector.memset(vdst[:, :, DH:VW], 1.0)

            # ---- phase 1.5: RoPE on Q^T and K^T (in place)
            for dst in (qT_sb, kT_sb):
                for mt in range(PAIRS):
                    t = dst[:, mt, :]
                    swp = wp.tile([128, L], BF16, tag="swp", bufs=2)
                    for i in range(4):
                        j = i ^ 1
                        nc.sync.dma_start(
                            swp[i * 32 : (i + 1) * 32, :], t[j * 32 : (j + 1) * 32, :]
                        )
                    nc.vector.tensor_mul(swp[:], swp[:], sin_sb[:])
                    nc.vector.tensor_mul(t, t, cos_sb[:])
                    nc.vector.tensor_add(t, t, swp[:])

            # ---- phase 2: attention, pair-granular.
            # Deferred-work queue: div-finish is split into A (reciprocal,
            # DVE-only) and B (broadcast matmul + normalize multiply), and
            # the output projection of chunk c is deferred into chunk c+1.
            # One item is drained per pair boundary, so every PE-visible
            # dependency (recip for B, muls for proj) has a full pair's
            # kt-loop (~5-10us) to complete first -> no PE stalls, HAM warm.
            pending = []

            def drain_pending(n):
                for _ in range(min(n, len(pending))):
                    pending.pop(0)()

            def make_proj(c):
                def proj():
                    for lt in range(4 * c, 4 * c + 4):
                        for cc in range(2):
                            op = ps.tile([128, CHUNK], F32, tag="qp",
                                         name=f"op_{lt}_{cc}")
                            for pr in range(PAIRS):
                                nc.tensor.matmul(
                                    op[:],
                                    yT_sb[:, pr, lt * 128 : (lt + 1) * 128],
                                    wo_sb[:, pr, cc * CHUNK : (cc + 1) * CHUNK],
                                    start=(pr == 0),
                                    stop=(pr == PAIRS - 1),
                                )
                            ob = wp.tile([128, CHUNK], F32, tag="ob", bufs=2)
                            nc.vector.tensor_copy(ob[:], op[:])
                            nc.sync.dma_start(
                                out[
                                    lt * 128 : (lt + 1) * 128,
                                    cc * CHUNK : (cc + 1) * CHUNK,
                                ],
                                ob[:],
                            )
                return proj

            for c in range(NCH):
                q0 = c * CHUNK
                n_lk = 4 * (c + 1)
                half_state = {}
                for pr in range(PAIRS):
                    ys = [
                        ps.tile([128, CHUNK], F32, tag="ys",
                                name=f"ys_{c}_{pr}_{hh}")
                        for hh in range(2)
                    ]
                    for kt in range(n_lk):
                        k0 = kt * 128
                        off = max(0, k0 - q0)
                        sps = ps.tile([128, 1024], F32, tag="sc",
                                      name=f"sps_{c}_{pr}_{kt}")
                        diag = k0 >= q0
                        for hh in range(2):
                            nc.tensor.matmul(
                                sps[:, hh * 512 : hh * 512 + 512],
                                kT_sb[hh * 64 : (hh + 1) * 64, pr, k0 : k0 + 128],
                                qT_sb[hh * 64 : (hh + 1) * 64, pr, q0 : q0 + CHUNK],
                                start=True,
                                stop=not diag,
                            )
                        if diag:
                            # accumulate -1e30 on the masked staircase via an
                            # identity-lhsT matmul; exp then gives exact 0.
                            for hh in range(2):
                                nc.tensor.matmul(
                                    sps[:, hh * 512 + off : hh * 512 + off + 128],
                                    id_sb[:],
                                    tri_sb[:],
                                    start=False,
                                    stop=True,
                                )
                        ex = wp.tile([128, 1024], BF16, tag="ex", bufs=4)
                        nc.scalar.activation(
                            ex[:], sps[:], mybir.ActivationFunctionType.Exp,
                            scale=float(1.0 / np.sqrt(DH)),
                        )
                        for hh in range(2):
                            h = 2 * pr + hh
                            nc.tensor.matmul(
                                ys[hh][0:VW, off:CHUNK],
                                v_sb[:, kt, h * VW : (h + 1) * VW],
                                ex[:, hh * 512 + off : (hh + 1) * 512],
                                start=(kt == 0),
                                stop=(kt == n_lk - 1),
                            )
                    # drain PV psum to SBUF fast (frees banks; PE rolls on)
                    if pr % 2 == 0:
                        stage = wp.tile([128, CHUNK], F32, tag="stage", bufs=2,
                                        name=f"stage_{c}_{pr // 2}")
                        half_state = {"stage": stage, "tiles": []}
                    for hh in range(2):
                        ya = wp.tile([65, CHUNK], F32, tag="ya", bufs=10,
                                     name=f"ya_{c}_{pr}_{hh}")
                        nc.vector.tensor_copy(ya[:], ys[hh][0:VW, :])
                        idx = (pr % 2) * 2 + hh
                        nc.sync.dma_start(
                            half_state["stage"][32 * idx : 32 * idx + 1, :],
                            ya[64:65, :],
                        )
                        half_state["tiles"].append((pr, hh, ya, idx))
                    if pr % 2 == 1:
                        st = half_state
                        stash = {}

                        def finishA(st=st, stash=stash):
                            stage_r = wp.tile([128, CHUNK], F32R, tag="str",
                                              bufs=2)
                            with nc.allow_low_precision(reason="f32r recip"):
                                nc.vector.reciprocal(stage_r[:], st["stage"][:])
                            stash["stage_r"] = stage_r

                        def finishB(st=st, stash=stash, q0=q0):
                            stage_r = stash["stage_r"]
                            for pr2, hh, ya, idx in st["tiles"]:
                                bc = ps.tile([128, CHUNK], F32, tag="qp",
                                             name=f"bc_{q0}_{pr2}_{hh}")
                                nc.tensor.matmul(
                                    bc[0:64, :],
                                    ones_sb[32 * idx : 32 * idx + 1, :],
                                    stage_r[32 * idx : 32 * idx + 1, :],
                                    start=True,
                                    stop=True,
                                    tile_position=(32 * idx, 0),
                                )
                                if hh == 0:
                                    nc.vector.tensor_mul(
                                        yT_sb[0:64, pr2, q0 : q0 + CHUNK],
                                        ya[0:64, :],
                                        bc[0:64, :],
                                    )
                                else:
                                    # elementwise out/in partition bases must
                                    # match; base-0 tmp + DMA moves to 64:128.
                                    yt = wp.tile([64, CHUNK], BF16, tag="yt",
                                                 bufs=2)
                                    nc.vector.tensor_mul(
                                        yt[:], ya[0:64, :], bc[0:64, :]
                                    )
                                    nc.sync.dma_start(
                                        yT_sb[64:128, pr2, q0 : q0 + CHUNK],
                                        yt[:],
                                    )

                        pending.append(finishA)
                        pending.append(finishB)
                    drain_pending(1)

                # chunk's projection is deferred into the next chunk's pair
                # boundaries; the final flush below emits the leftovers.
                pending.append(make_proj(c))
            drain_pending(len(pending))
    return nc


def _rope_tables():
    inv_freq = (1.0 / (10000.0 ** (np.arange(0, DH, 2, dtype=np.float32) / DH))).astype(
        np.float32
    )
    t = np.arange(L, dtype=np.float32)
    freqs = np.einsum("l,d->ld", t, inv_freq).astype(np.float32)  # (L, 32)
    emb = np.concatenate([freqs, freqs], axis=-1)                 # (L, 64)
    cos = np.cos(emb).astype(np.float32)
    sin = np.sin(emb).astype(np.float32)
    cosT = cos.T                                   # (64, L)
    sinT = sin.T.copy()
    sinT[0:32] = -sinT[0:32]                       # fold rotate_half sign
    cos128 = np.tile(cosT, (2, 1))                 # (128, L)
    sin128 = np.tile(sinT, (2, 1))
    return cos128, sin128


def _tri_neg():
    # trin[p, t] = NEG where key-partition p is masked for query-col t of the
    # staircase window (p > t), else 0.
    p = np.arange(128)[:, None]
    t = np.arange(128)[None, :]
    return np.where(p > t, NEG, 0.0).astype(np.float32)


def _bf16(a):
    return np.asarray(a, dtype=np.float32).astype(ml_dtypes.bfloat16)


_COMPILED = None


def kernel(x, pad_mask, W_qkv, b_qkv, W_out, b_out):
    global LAST_RESULTS, _COMPILED
    from concourse.bass_utils import run_bass_kernel_spmd

    x = np.asarray(x, dtype=np.float32)
    W_qkv = np.asarray(W_qkv, dtype=np.float32)
    b_qkv = np.asarray(b_qkv, dtype=np.float32)
    W_out = np.asarray(W_out, dtype=np.float32)
    b_out = np.asarray(b_out, dtype=np.float32)

    cos128, sin128 = _rope_tables()

    in_maps = []
    for core in range(NCORES):
        b, g = core // G, core % G
        sl = slice(g * DQ, (g + 1) * DQ)
        wqv = W_qkv[:, 0 * D : 1 * D][:, sl]
        wkv = W_qkv[:, 1 * D : 2 * D][:, sl]
        wvv = W_qkv[:, 2 * D : 3 * D][:, sl]
        bqv = b_qkv[0 * D : 1 * D][sl]
        bkv = b_qkv[1 * D : 2 * D][sl]
        bvv = b_qkv[2 * D : 3 * D][sl]
        in_maps.append(
            {
                "xT": _bf16(x[b].T),
                "wq": _bf16(wqv),
                "wk": _bf16(wkv),
                "wv": _bf16(wvv),
                "wo": _bf16(W_out[sl, :]),
                "bq": np.ascontiguousarray(bqv.reshape(PAIRS, 128).T),
                "bk": np.ascontiguousarray(bkv.reshape(PAIRS, 128).T),
                "bv": np.tile(bvv[None, :], (128, 1)).astype(np.float32),
                "cosT": _bf16(cos128),
                "sinT": _bf16(sin128),
                "trin": _bf16(_tri_neg()),
                "iden": _bf16(np.eye(128, dtype=np.float32)),
            }
        )

    if _COMPILED is None:
        nc = build_module()
        fixed = legalize_bir_waits(nc.to_json_bytes())
        nc.to_json_bytes = lambda: fixed  # bass2jax ships this BIR to walrus
        _COMPILED = nc
    nc = _COMPILED

    res = run_bass_kernel_spmd(
        nc,
        in_maps,
        core_ids=list(range(NCORES)),
        trace=bool(os.environ.get("BASS_TRACE")),
    )
    LAST_RESULTS = res

    out = np.zeros((B, L, D), dtype=np.float32)
    for core in range(NCORES):
        out[core // G] += np.asarray(res.results[core]["out"], dtype=np.float32)
    out += b_out[None, None, :]
    return out


# revision 5
# speedup vs baseline: 1.9100x; 1.1857x over previous
"""Causal self-attention with RoPE on 8 Trainium2 NeuronCores.

Sharding: batch x head-group. Core c handles batch b = c//2 and head group
g = c%2 (8 of 16 heads). Each core runs the full per-(batch, head-group)
pipeline on device:

  QKV^T projection -> RoPE -> causal flash-style attention -> partial
  output projection (its heads' slice of W_out rows).

The host sums the two partial projections per batch and adds b_out.

v3 pipeline (v2 measured 414us: attention phase ACT-bound at ~95% Scalar
busy while PE idled ~40%, QKV phase the reverse): QKV projection + RoPE
for chunk c+1 are emitted INTO the PE-idle slots between attention pairs
of chunk c (the PE queue is in-order, so placement in program order is
what fills the gaps).  Per-pair boundary schedule for chunk c:
  b0: div-finishB(half1, c-1) | qkv-unit(c+1, mt0) | v-unit(c+1, 0)
  b1: div-finishA(half0, c)   | proj(c-1)          | qkv(c+1, mt1) | v(..1)
  b2: div-finishB(half0, c)   | qkv(c+1, mt2)      | v(..2)
  b3: div-finishA(half1, c)   | qkv(c+1, mt3)      | v(..3)
finishA (the 4us batched reciprocal) always lands one full pair kt-loop
before the finishB that consumes it, and proj(c) one boundary after the
final normalize-mul of chunk c, so no PE-visible dependency ever stalls.

Other v3 changes: QKV biases moved ACT->DVE (tensor_scalar_add), exp
ACTIVATEs narrowed past the causal diagonal via a strided AP, output
written bf16 (halves the 8MB/core output DMA).
"""

import os
import sys

if "/opt/trn_rl_repo" not in sys.path:
    sys.path.insert(0, "/opt/trn_rl_repo")

import numpy as np
import ml_dtypes

import concourse.bass as bass
import concourse.mybir as mybir
import concourse.tile as tile

F32 = mybir.dt.float32
F32R = mybir.dt.float32r
BF16 = mybir.dt.bfloat16

B, L, D = 4, 2048, 1024
H, DH = 16, 64
NCORES = 8
G = 2                 # head groups (cores per batch)
HPC = H // G          # heads per core = 8
DQ = HPC * DH         # per-core q/k/v width = 512
PAIRS = HPC // 2      # 128-partition head pairs = 4
CHUNK = 512           # query-chunk (matmul free dim)
NCH = L // CHUNK      # 4
KT = D // 128         # 8 k-tiles over d_model
LT = L // 128         # 16 l-tiles
VW = DH + 1           # V columns per head incl. ones column = 65
NEG = -1.0e30         # causal-mask additive constant (exp -> exact 0)

LAST_RESULTS = None   # test harness reads perf fields from here


def legalize_bir_waits(bir_json: bytes) -> bytes:
    """Split multi-wait sync_infos into standalone EventSemaphore instrs.

    This container's walrus codegen accepts at most ONE sync wait per
    instruction (two for EventSemaphore), but Tile's sem assigner happily
    attaches several.  For every instruction carrying N>1 waits, keep one
    and hoist the rest onto EventSemaphore instructions inserted directly
    before it on the same engine (same block), which preserves each
    engine's program order and therefore the sync semantics.
    """
    import json as _json

    j = _json.loads(bir_json)
    uid = [0]
    for fn in j["functions"]:
        for blk in fn["blocks"]:
            out_insts = []
            for inst in blk["instructions"]:
                si = inst.get("sync_info")
                waits = (si or {}).get("on_wait") or []
                cap = 2 if inst.get("opcode") == "EventSemaphore" else 1
                if len(waits) > cap:
                    extra, keep = waits[:-cap], waits[-cap:]
                    for i in range(0, len(extra), 2):
                        uid[0] += 1
                        out_insts.append(
                            {
                                "name": f"antwaitfix-{uid[0]}",
                                "opcode": "EventSemaphore",
                                "engine": inst["engine"],
                                "ins": [],
                                "outs": [],
                                "debug": inst.get("debug", 0),
                                "sync_info": {
                                    "on_wait": extra[i : i + 2],
                                    "on_update": [],
                                },
                            }
                        )
                    si["on_wait"] = keep
                out_insts.append(inst)
            blk["instructions"] = out_insts
    return _json.dumps(j).encode()


def build_module():
    nc = bass.Bass(use_seq_codegen=True)

    xT = nc.declare_dram_parameter("xT", [D, L], BF16, isOutput=False)
    wq = nc.declare_dram_parameter("wq", [D, DQ], BF16, isOutput=False)
    wk = nc.declare_dram_parameter("wk", [D, DQ], BF16, isOutput=False)
    wv = nc.declare_dram_parameter("wv", [D, DQ], BF16, isOutput=False)
    wo = nc.declare_dram_parameter("wo", [DQ, D], BF16, isOutput=False)
    bq = nc.declare_dram_parameter("bq", [128, PAIRS], F32, isOutput=False)
    bk = nc.declare_dram_parameter("bk", [128, PAIRS], F32, isOutput=False)
    bv = nc.declare_dram_parameter("bv", [128, DQ], F32, isOutput=False)
    cosT = nc.declare_dram_parameter("cosT", [128, L], BF16, isOutput=False)
    sinT = nc.declare_dram_parameter("sinT", [128, L], BF16, isOutput=False)
    trin = nc.declare_dram_parameter("trin", [128, 128], BF16, isOutput=False)
    iden = nc.declare_dram_parameter("iden", [128, 128], BF16, isOutput=False)
    out = nc.declare_dram_parameter("out", [L, D], BF16, isOutput=True)

    with tile.TileContext(nc) as tc:
        with (
            tc.tile_pool(name="const", bufs=1) as cp,
            tc.tile_pool(name="acts", bufs=1) as ap,
            tc.tile_pool(name="work", bufs=2) as wp,
            tc.tile_pool(name="psum", bufs=2, space="PSUM") as ps,
        ):
            # ---- constant / activation loads (split for DMA-queue spread)
            xT_sb = ap.tile([128, KT, L], BF16)
            for kt in range(KT):
                nc.sync.dma_start(
                    xT_sb[:, kt, :],
                    xT.rearrange("(kt p) l -> p kt l", p=128)[:, kt, :],
                )
            wq_sb = cp.tile([128, KT, DQ], BF16)
            wk_sb = cp.tile([128, KT, DQ], BF16)
            wv_sb = cp.tile([128, KT, DQ], BF16)
            for kt in range(KT):
                nc.sync.dma_start(
                    wq_sb[:, kt, :], wq.rearrange("(kt p) m -> p kt m", p=128)[:, kt, :]
                )
                nc.sync.dma_start(
                    wk_sb[:, kt, :], wk.rearrange("(kt p) m -> p kt m", p=128)[:, kt, :]
                )
                nc.sync.dma_start(
                    wv_sb[:, kt, :], wv.rearrange("(kt p) m -> p kt m", p=128)[:, kt, :]
                )
            wo_sb = cp.tile([128, PAIRS, D], BF16)
            for pr in range(PAIRS):
                nc.sync.dma_start(
                    wo_sb[:, pr, :], wo.rearrange("(pr p) c -> p pr c", p=128)[:, pr, :]
                )
            bq_sb = cp.tile([128, PAIRS], F32)
            bk_sb = cp.tile([128, PAIRS], F32)
            bv_sb = cp.tile([128, DQ], F32)
            cos_sb = cp.tile([128, L], BF16)
            sin_sb = cp.tile([128, L], BF16)
            tri_sb = cp.tile([128, 128], BF16)
            id_sb = cp.tile([128, 128], BF16)
            nc.sync.dma_start(bq_sb[:], bq[:])
            nc.sync.dma_start(bk_sb[:], bk[:])
            nc.sync.dma_start(bv_sb[:], bv[:])
            nc.sync.dma_start(cos_sb[:], cosT[:])
            nc.sync.dma_start(sin_sb[:], sinT[:])
            nc.sync.dma_start(tri_sb[:], trin[:])
            nc.sync.dma_start(id_sb[:], iden[:])
            # memset can't encode a float32r immediate; memset f32 then
            # copy-convert (bitwise identical) into the f32r tile.
            ones_f32 = cp.tile([128, 64], F32)
            nc.vector.memset(ones_f32[:], 1.0)
            ones_sb = cp.tile([128, 64], F32R)
            with nc.allow_low_precision(reason="f32r ones for bcast mm"):
                nc.vector.tensor_copy(ones_sb[:], ones_f32[:])

            qT_sb = ap.tile([128, PAIRS, L], BF16)
            kT_sb = ap.tile([128, PAIRS, L], BF16)
            v_sb = ap.tile([128, LT, HPC * VW], BF16)
            yT_sb = ap.tile([128, PAIRS, L], BF16)

            # ---------------- emission units ----------------
            def qkv_unit(c, mt):
                """q+k projection, bias, and RoPE for (pair mt, chunk c)."""
                q0 = c * CHUNK
                qk = ps.tile([128, 1024], F32, tag="sc", name=f"qk_{c}_{mt}")
                for half, w_sb in ((0, wq_sb), (1, wk_sb)):
                    for kt in range(KT):
                        nc.tensor.matmul(
                            qk[:, half * 512 : half * 512 + 512],
                            w_sb[:, kt, mt * 128 : (mt + 1) * 128],
                            xT_sb[:, kt, q0 : q0 + CHUNK],
                            start=(kt == 0),
                            stop=(kt == KT - 1),
                        )
                nc.vector.tensor_scalar_add(
                    qT_sb[:, mt, q0 : q0 + CHUNK], qk[:, 0:512],
                    bq_sb[:, mt : mt + 1],
                )
                nc.vector.tensor_scalar_add(
                    kT_sb[:, mt, q0 : q0 + CHUNK], qk[:, 512:1024],
                    bk_sb[:, mt : mt + 1],
                )
                # RoPE in place on this (pair, chunk) slice
                for dst in (qT_sb, kT_sb):
                    t = dst[:, mt, q0 : q0 + CHUNK]
                    swp = wp.tile([128, CHUNK], BF16, tag="swp", bufs=2)
                    for i in range(4):
                        j = i ^ 1
                        nc.sync.dma_start(
                            swp[i * 32 : (i + 1) * 32, :],
                            t[j * 32 : (j + 1) * 32, :],
                        )
                    nc.vector.tensor_mul(
                        swp[:], swp[:], sin_sb[:, q0 : q0 + CHUNK]
                    )
                    nc.vector.tensor_mul(t, t, cos_sb[:, q0 : q0 + CHUNK])
                    nc.vector.tensor_add(t, t, swp[:])

            def v_unit(lt):
                """V projection (+bias, ones column) for l-tile lt."""
                vps = ps.tile([128, CHUNK], F32, tag="qp", name=f"vps_{lt}")
                for kt in range(KT):
                    nc.tensor.matmul(
                        vps[:],
                        xT_sb[:, kt, lt * 128 : (lt + 1) * 128],
                        wv_sb[:, kt, :],
                        start=(kt == 0),
                        stop=(kt == KT - 1),
                    )
                vdst = v_sb[:, lt, :].rearrange("p (h c) -> p h c", c=VW)
                nc.vector.tensor_add(vdst[:, :, 0:DH], vps[:], bv_sb[:])
                nc.vector.memset(vdst[:, :, DH:VW], 1.0)

            def make_proj(c):
                def proj():
                    for lt in range(4 * c, 4 * c + 4):
                        for cc in range(2):
                            op = ps.tile([128, CHUNK], F32, tag="qp",
                                         name=f"op_{lt}_{cc}")
                            for pr in range(PAIRS):
                                nc.tensor.matmul(
                                    op[:],
                                    yT_sb[:, pr, lt * 128 : (lt + 1) * 128],
                                    wo_sb[:, pr, cc * CHUNK : (cc + 1) * CHUNK],
                                    start=(pr == 0),
                                    stop=(pr == PAIRS - 1),
                                )
                            ob = wp.tile([128, CHUNK], BF16, tag="ob", bufs=2)
                            nc.vector.tensor_copy(ob[:], op[:])
                            nc.sync.dma_start(
                                out[
                                    lt * 128 : (lt + 1) * 128,
                                    cc * CHUNK : (cc + 1) * CHUNK,
                                ],
                                ob[:],
                            )
                return proj

            # ---- pipeline prologue: chunk-0 QKV + RoPE + V
            for mt in range(PAIRS):
                qkv_unit(0, mt)
            for lt in range(4):
                v_unit(lt)

            # ---- attention pipeline
            pending = []           # staggered div-finish / proj closures

            def drain_pending(n):
                for _ in range(min(n, len(pending))):
                    pending.pop(0)()

            for c in range(NCH):
                q0 = c * CHUNK
                n_lk = 4 * (c + 1)
                half_state = {}
                for pr in range(PAIRS):
                    ys = [
                        ps.tile([128, CHUNK], F32, tag="ys",
                                name=f"ys_{c}_{pr}_{hh}")
                        for hh in range(2)
                    ]
                    for kt in range(n_lk):
                        k0 = kt * 128
                        off = max(0, k0 - q0)
                        sps = ps.tile([128, 1024], F32, tag="sc",
                                      name=f"sps_{c}_{pr}_{kt}")
                        diag = k0 >= q0
                        for hh in range(2):
                            nc.tensor.matmul(
                                sps[:, hh * 512 : hh * 512 + 512],
                                kT_sb[hh * 64 : (hh + 1) * 64, pr, k0 : k0 + 128],
                                qT_sb[hh * 64 : (hh + 1) * 64, pr, q0 : q0 + CHUNK],
                                start=True,
                                stop=not diag,
                            )
                        if diag:
                            # accumulate -1e30 on the masked staircase via an
                            # identity-lhsT matmul; exp then gives exact 0.
                            for hh in range(2):
                                nc.tensor.matmul(
                                    sps[:, hh * 512 + off : hh * 512 + off + 128],
                                    id_sb[:],
                                    tri_sb[:],
                                    start=False,
                                    stop=True,
                                )
                        ex = wp.tile([128, 1024], BF16, tag="ex", bufs=4)
                        if off:
                            # skip the fully-masked columns left of the
                            # staircase (strided 2-window AP)
                            nc.scalar.activation(
                                ex.rearrange("p (h w) -> p h w", w=512)[:, :, off:],
                                sps.rearrange("p (h w) -> p h w", w=512)[:, :, off:],
                                mybir.ActivationFunctionType.Exp,
                                scale=float(1.0 / np.sqrt(DH)),
                            )
                        else:
                            nc.scalar.activation(
                                ex[:], sps[:], mybir.ActivationFunctionType.Exp,
                                scale=float(1.0 / np.sqrt(DH)),
                            )
                        for hh in range(2):
                            h = 2 * pr + hh
                            nc.tensor.matmul(
                                ys[hh][0:VW, off:CHUNK],
                                v_sb[:, kt, h * VW : (h + 1) * VW],
                                ex[:, hh * 512 + off : (hh + 1) * 512],
                                start=(kt == 0),
                                stop=(kt == n_lk - 1),
                            )
                    # drain PV psum to SBUF fast (frees banks; PE rolls on)
                    if pr % 2 == 0:
                        stage = wp.tile([128, CHUNK], F32, tag="stage", bufs=2,
                                        name=f"stage_{c}_{pr // 2}")
                        half_state = {"stage": stage, "tiles": []}
                    for hh in range(2):
                        ya = wp.tile([65, CHUNK], F32, tag="ya", bufs=10,
                                     name=f"ya_{c}_{pr}_{hh}")
                        nc.vector.tensor_copy(ya[:], ys[hh][0:VW, :])
                        idx = (pr % 2) * 2 + hh
                        nc.sync.dma_start(
                            half_state["stage"][32 * idx : 32 * idx + 1, :],
                            ya[64:65, :],
                        )
                        half_state["tiles"].append((pr, hh, ya, idx))
                    if pr % 2 == 1:
                        st = half_state
                        stash = {}

                        def finishA(st=st, stash=stash):
                            stage_r = wp.tile([128, CHUNK], F32R, tag="str",
                                              bufs=2)
                            with nc.allow_low_precision(reason="f32r recip"):
                                nc.vector.reciprocal(stage_r[:], st["stage"][:])
                            stash["stage_r"] = stage_r

                        def finishB(st=st, stash=stash, q0=q0):
                            stage_r = stash["stage_r"]
                            for pr2, hh, ya, idx in st["tiles"]:
                                bc = ps.tile([128, CHUNK], F32, tag="qp",
                                             name=f"bc_{q0}_{pr2}_{hh}")
                                nc.tensor.matmul(
                                    bc[0:64, :],
                                    ones_sb[32 * idx : 32 * idx + 1, :],
                                    stage_r[32 * idx : 32 * idx + 1, :],
                                    start=True,
                                    stop=True,
                                    tile_position=(32 * idx, 0),
                                )
                                if hh == 0:
                                    nc.vector.tensor_mul(
                                        yT_sb[0:64, pr2, q0 : q0 + CHUNK],
                                        ya[0:64, :],
                                        bc[0:64, :],
                                    )
                                else:
                                    # elementwise out/in partition bases must
                                    # match; base-0 tmp + DMA moves to 64:128.
                                    yt = wp.tile([64, CHUNK], BF16, tag="yt",
                                                 bufs=2)
                                    nc.vector.tensor_mul(
                                        yt[:], ya[0:64, :], bc[0:64, :]
                                    )
                                    nc.sync.dma_start(
                                        yT_sb[64:128, pr2, q0 : q0 + CHUNK],
                                        yt[:],
                                    )

                        pending.append(finishA)
                        pending.append(finishB)
                    # ---- pair boundary: one staggered div/proj item + the
                    # next chunk's QKV/V work to fill PE's exp-wait slots
                    drain_pending(1)
                    if c + 1 < NCH:
                        qkv_unit(c + 1, pr)
                        v_unit(4 * (c + 1) + pr)

                pending.append(make_proj(c))
            drain_pending(len(pending))
    return nc


def _rope_tables():
    inv_freq = (1.0 / (10000.0 ** (np.arange(0, DH, 2, dtype=np.float32) / DH))).astype(
        np.float32
    )
    t = np.arange(L, dtype=np.float32)
    freqs = np.einsum("l,d->ld", t, inv_freq).astype(np.float32)  # (L, 32)
    emb = np.concatenate([freqs, freqs], axis=-1)                 # (L, 64)
    cos = np.cos(emb).astype(np.float32)
    sin = np.sin(emb).astype(np.float32)
    cosT = cos.T                                   # (64, L)
    sinT = sin.T.copy()
    sinT[0:32] = -sinT[0:32]                       # fold rotate_half sign
    cos128 = np.tile(cosT, (2, 1))                 # (128, L)
    sin128 = np.tile(sinT, (2, 1))
    return cos128, sin128


def _tri_neg():
    # trin[p, t] = NEG where key-partition p is masked for query-col t of the
    # staircase window (p > t), else 0.
    p = np.arange(128)[:, None]
    t = np.arange(128)[None, :]
    return np.where(p > t, NEG, 0.0).astype(np.float32)


def _bf16(a):
    return np.asarray(a, dtype=np.float32).astype(ml_dtypes.bfloat16)


_COMPILED = None


def kernel(x, pad_mask, W_qkv, b_qkv, W_out, b_out):
    global LAST_RESULTS, _COMPILED
    from concourse.bass_utils import run_bass_kernel_spmd

    x = np.asarray(x, dtype=np.float32)
    W_qkv = np.asarray(W_qkv, dtype=np.float32)
    b_qkv = np.asarray(b_qkv, dtype=np.float32)
    W_out = np.asarray(W_out, dtype=np.float32)
    b_out = np.asarray(b_out, dtype=np.float32)

    cos128, sin128 = _rope_tables()

    in_maps = []
    for core in range(NCORES):
        b, g = core // G, core % G
        sl = slice(g * DQ, (g + 1) * DQ)
        wqv = W_qkv[:, 0 * D : 1 * D][:, sl]
        wkv = W_qkv[:, 1 * D : 2 * D][:, sl]
        wvv = W_qkv[:, 2 * D : 3 * D][:, sl]
        bqv = b_qkv[0 * D : 1 * D][sl]
        bkv = b_qkv[1 * D : 2 * D][sl]
        bvv = b_qkv[2 * D : 3 * D][sl]
        in_maps.append(
            {
                "xT": _bf16(x[b].T),
                "wq": _bf16(wqv),
                "wk": _bf16(wkv),
                "wv": _bf16(wvv),
                "wo": _bf16(W_out[sl, :]),
                "bq": np.ascontiguousarray(bqv.reshape(PAIRS, 128).T),
                "bk": np.ascontiguousarray(bkv.reshape(PAIRS, 128).T),
                "bv": np.tile(bvv[None, :], (128, 1)).astype(np.float32),
                "cosT": _bf16(cos128),
                "sinT": _bf16(sin128),
                "trin": _bf16(_tri_neg()),
                "iden": _bf16(np.eye(128, dtype=np.float32)),
            }
        )

    if _COMPILED is None:
        nc = build_module()
        fixed = legalize_bir_waits(nc.to_json_bytes())
        nc.to_json_bytes = lambda: fixed  # bass2jax ships this BIR to walrus
        _COMPILED = nc
    nc = _COMPILED

    res = run_bass_kernel_spmd(
        nc,
        in_maps,
        core_ids=list(range(NCORES)),
        trace=bool(os.environ.get("BASS_TRACE")),
    )
    LAST_RESULTS = res

    out = np.zeros((B, L, D), dtype=np.float32)
    for core in range(NCORES):
        out[core // G] += np.asarray(res.results[core]["out"], dtype=np.float32)
    out += b_out[None, None, :]
    return out
